# revision 1
# baseline (speedup 1.0000x reference)
"""Trainium2 Bass kernel for nn_BasicBlock (Minkowski sparse-conv basic block).

Strategy (8 NeuronCores, SPMD):
- Points N=400000 dest-sharded: core c owns output rows [c*50000,(c+1)*50000).
- Host routes messages: for each conv, msgs (k,e) grouped per (core, window of
  128 output rows, k). Each window has a uniform layout: 27 k-runs of 64 lanes
  (overflow msgs are pre-multiplied by W_k on host and placed in an identity-
  weight spill region), so one SPMD program serves all cores.
- Device per window: stream tile in (bf16) -> PE pack-2 transpose -> per-k
  matmul vs W_k (R-form, 32-aligned runs) -> one-hot P (is_equal vs iota) ->
  scatter matmul P^T @ msg accumulating the [128,64] window in PSUM -> flush.
- Instance-norm stats via ones-vector matmuls; AllReduce across cores for
  norm2; norm1 applied host-side when building the conv2 stream (h = relu(
  a1*y1+b1) feeds conv2's gather). Residual + final relu fused on device.
"""
import numpy as np
import ml_dtypes

N, C = 400000, 64
K, E = 27, 200000
EPS = 1e-5
NCORES = 8
SHARD = N // NCORES          # 50000
WIN = 128
NW = (SHARD + WIN - 1) // WIN  # 391
PADROWS = NW * WIN             # 50048
R = 64                         # lanes per k-run
BASE_LANES = K * R             # 1728

BF16 = ml_dtypes.bfloat16

_cache = {}


def _route_conv(out_idx_flat, k_flat):
    """Per (core, window, k) routing. Returns per-core dicts of lane tables."""
    core = out_idx_flat // SHARD
    rowpos = out_idx_flat - core * SHARD
    win = rowpos // WIN
    loc = rowpos - win * WIN
    return core, win, loc


def _build_stream(src_rows, core, win, loc, k_flat, Wk, n_blocks_min=14):
    """Build per-core lane tables.

    src_rows: [M,64] float32 message *inputs* (rows to be multiplied by W_k),
    core/win/loc/k_flat: [M] routing. Wk: [K,64,64].
    Returns list per core: (stream [128, B*NW*64] bf16, oiT [128, B*NW] bf16, B)
    Lane layout per window: lanes [k*64,(k+1)*64) = first 64 msgs of k;
    overflow lanes at [BASE_LANES, B*128) hold host-premultiplied msgs.
    """
    M = src_rows.shape[0]
    # rank of each msg within its (core,win,k) cell
    cell = (core.astype(np.int64) * NW + win) * K + k_flat
    order = np.argsort(cell, kind="stable")
    cell_s = cell[order]
    # rank within cell
    uniq, starts = np.unique(cell_s, return_index=True)
    rank = np.arange(M, dtype=np.int64)
    rank -= np.repeat(starts, np.diff(np.append(starts, M)))
    inrun = rank < R
    # spill ranks per (core,win)
    cw_s = cell_s // K
    spill_mask = ~inrun
    spill_cw = cw_s[spill_mask]
    so = np.argsort(spill_cw, kind="stable")
    su, sstarts = np.unique(spill_cw[so], return_index=True)
    srank = np.arange(spill_cw.shape[0], dtype=np.int64)
    srank -= np.repeat(sstarts, np.diff(np.append(sstarts, spill_cw.shape[0])))
    # spill capacity -> B
    max_spill = int(srank.max()) + 1 if srank.size else 0
    B = max(n_blocks_min, (BASE_LANES + max_spill + 127) // 128)
    LPW = B * 128
    # lane index within window
    lane = np.where(inrun, (cell_s % K) * R + rank, 0)
    lane_sp = np.zeros(M, np.int64)
    lane_sp_vals = BASE_LANES + srank
    tmp = np.zeros(spill_cw.shape[0], np.int64)
    tmp[so] = lane_sp_vals
    lane_sp[spill_mask] = tmp
    lane = np.where(inrun, lane, lane_sp)
    assert lane.max() < LPW

    rows = src_rows[order].astype(np.float32)
    kk = k_flat[order]
    # premultiply spill rows by their W_k
    if spill_mask.any():
        sm = spill_mask
        rows[sm] = np.einsum("mc,mcd->md", rows[sm], Wk[kk[sm]]).astype(np.float32)

    cores_out = []
    core_s = cw_s // NW
    win_s = cw_s % NW
    loc_s = loc[order]
    for c in range(NCORES):
        m = core_s == c
        gl = win_s[m] * LPW + lane[m]           # global lane in this core
        stream = np.zeros((NW * LPW, C), np.float32)
        stream[gl] = rows[m]
        oi = np.full(NW * LPW, -1.0, np.float32)
        oi[gl] = loc_s[m].astype(np.float32)
        # lane-major layouts: [128, nblk*64] and [128, nblk]
        nblk = NW * B
        stream_lm = np.ascontiguousarray(
            stream.reshape(nblk, 128, C).transpose(1, 0, 2).reshape(128, nblk * C)
        ).astype(BF16)
        oi_lm = np.ascontiguousarray(
            oi.reshape(nblk, 128).transpose(1, 0)
        ).astype(np.float32)
        cores_out.append((stream_lm, oi_lm))
    return cores_out, B


def _w_table(Wk):
    """[128, 28*64] f32: rows 0-63 and 64-127 both hold [W_0..W_26, I]."""
    wt = np.zeros((128, (K + 1) * C), np.float32)
    flat = np.concatenate([Wk.transpose(0, 1, 2).reshape(K * C, C),
                           np.eye(C, dtype=np.float32)], axis=0)  # [(K+1)*64,64]
    w2 = flat.reshape(K + 1, C, C)
    for k in range(K + 1):
        wt[0:64, k * C:(k + 1) * C] = w2[k]
        wt[64:128, k * C:(k + 1) * C] = w2[k]
    return wt


def _build_program(B, NW, NTOT, with_norm_out, ncores=8):
    """Build the SPMD Bass program for one conv.

    B: blocks per window; NW: windows; NTOT: total rows for mean divisor.
    """
    from concourse import bass, bacc, tile, mybir
    from concourse.masks import make_identity

    F32 = mybir.dt.float32
    BF = mybir.dt.bfloat16
    ActF = mybir.ActivationFunctionType
    Alu = mybir.AluOpType

    nc = bacc.Bacc("TRN2", target_bir_lowering=False, debug=False,
                   num_devices=ncores)
    stream_d = nc.dram_tensor("stream", [128, NW * B * C], BF,
                              kind="ExternalInput")
    oi_d = nc.dram_tensor("oiT", [128, NW * B], F32, kind="ExternalInput")
    wt_d = nc.dram_tensor("wt", [128, (K + 1) * C], F32, kind="ExternalInput")
    iota_d = nc.dram_tensor("iota", [128, 128], BF, kind="ExternalInput")
    if with_norm_out:
        xr_d = nc.dram_tensor("xr", [128, NW * C], F32, kind="ExternalInput")
        gb_d = nc.dram_tensor("gb", [1, 4 * C], F32, kind="ExternalInput")
        out_d = nc.dram_tensor("out", [128, NW * C], F32, kind="ExternalOutput")
    y_d = nc.dram_tensor("y", [128, NW * C], F32, kind="ExternalOutput")
    stats_d = nc.dram_tensor("stats", [1, 2 * C], F32, kind="ExternalOutput")

    STAGE = int(__import__("os").environ.get("KSTAGE", "9"))
    NPACK = (B * C + 127) // 128      # pack-2 transposes per window
    NPT = (NPACK + 3) // 4            # psum transpose tiles [128,512]
    NMT = (B + 7) // 8                # msg psum tiles [128,512] (8 blocks each)

    with tile.TileContext(nc) as tc:
        with (
            tc.tile_pool(name="const", bufs=1) as constp,
            tc.tile_pool(name="sb", bufs=3) as sb,
            tc.tile_pool(name="msb", bufs=2) as msb,
            tc.tile_pool(name="tp", bufs=2, space="PSUM") as tpp,
            tc.tile_pool(name="mp", bufs=1, space="PSUM") as mpp,
            tc.tile_pool(name="yp", bufs=2, space="PSUM") as ypp,
            tc.tile_pool(name="statp", bufs=1, space="PSUM") as statp,
            tc.tile_pool(name="dram", bufs=1, space="DRAM") as dramp,
        ):
            identb = constp.tile([128, 128], BF)
            make_identity(nc, identb[:])
            iota_t = constp.tile([128, 128], BF)
            nc.sync.dma_start(iota_t[:], iota_d[:])
            w_t = constp.tile([128, (K + 1) * C], F32)
            nc.sync.dma_start(w_t[:], wt_d[:])
            wb_t = constp.tile([128, (K + 1) * C], BF)
            nc.vector.tensor_copy(wb_t[:], w_t[:])
            ones_col = constp.tile([128, 1], F32)
            nc.gpsimd.memset(ones_col[:], 1.0)

            stat_sum = statp.tile([1, C], F32, tag="ssum")
            stat_sq = statp.tile([1, C], F32, tag="ssq")

            for s in range(NW):
                st = sb.tile([128, B * C], BF, tag="stream")
                nc.sync.dma_start(st[:], stream_d[:, s * B * C:(s + 1) * B * C])
                oi_t = sb.tile([128, B], F32, tag="oi")
                nc.sync.dma_start(oi_t[:], oi_d[:, s * B:(s + 1) * B])

                # per-block transposes (channels at rows 0-63) into
                # [64,512] psum tiles (4 blocks each), copy to SBUF
                xgT = sb.tile([64, B * 128], BF, tag="xgT")
                NTT = (B + 3) // 4
                for pt in range(NTT):
                    lo_b = pt * 4
                    hi_b = min(B, lo_b + 4)
                    tps = tpp.tile([64, 512], BF, tag="tps")
                    for b in range(lo_b, hi_b):
                        nc.tensor.transpose(
                            out=tps[0:64, (b - lo_b) * 128:(b - lo_b) * 128 + 128],
                            in_=st[:, b * C:(b + 1) * C],
                            identity=identb[:],
                        )
                    cw = (hi_b - lo_b) * 128
                    dst = xgT[:, lo_b * 128:lo_b * 128 + cw]
                    if pt % 2 == 0:
                        nc.scalar.activation(dst, tps[:, 0:cw], ActF.Copy)
                    else:
                        nc.vector.tensor_copy(dst, tps[:, 0:cw])

                # mm1: 27 k-runs of 64 lanes + identity spill runs
                msgps = []
                for j in range(NMT):
                    mpt = mpp.tile([128, 512], F32, tag=f"mps{j}", name=f"mps{j}")
                    msgps.append(mpt)

                def mm1(lane0, cnt, wslice):
                    j = lane0 // 128
                    lo = lane0 % 128
                    nc.tensor.matmul(
                        out=msgps[j // 8][lo:lo + cnt, (j % 8) * C:(j % 8 + 1) * C],
                        lhsT=xgT[0:64, j * 128 + lo:j * 128 + lo + cnt],
                        rhs=wb_t[0:64, wslice * C:(wslice + 1) * C],
                        start=True, stop=True,
                        tile_position=(0, lo),
                    )

                for k in range(K if STAGE >= 2 else 0):
                    mm1(k * 64, 64, k)
                a = BASE_LANES if STAGE >= 2 else B * 128
                while a < B * 128:
                    blk, lo = a // 128, a % 128
                    cap = {0: 128, 32: 32, 64: 64, 96: 32}[lo]
                    e = min(B * 128, blk * 128 + lo + cap)
                    mm1(a, e - a, K)
                    a = e

                msg = msb.tile([128, B * C], BF, tag="msg")
                for j in range(NMT):
                    w = min(512, (B - j * 8) * C)
                    dst = msg[:, j * 512:j * 512 + w]
                    if j % 2 == 0:
                        nc.vector.tensor_copy(dst, msgps[j][:, 0:w])
                    else:
                        nc.scalar.activation(dst, msgps[j][:, 0:w], ActF.Copy)

                # P-gen + scatter matmul into the window accumulator
                ywin = ypp.tile([WIN, C], F32, tag="ywin")
                for b in range(B if STAGE >= 3 else 1):
                    P = sb.tile([128, WIN], BF, tag="P")
                    eng = nc.vector if b % 2 == 0 else nc.gpsimd
                    eng.tensor_scalar(
                        out=P[:], in0=iota_t[:], scalar1=oi_t[:, b:b + 1],
                        scalar2=None, op0=Alu.is_equal,
                    )
                    nc.tensor.matmul(
                        out=ywin[:], lhsT=P[:], rhs=msg[:, b * C:(b + 1) * C],
                        start=(b == 0), stop=(b == B - 1) or STAGE < 3,
                    )

                yst = msb.tile([WIN, C], F32, tag="yst")
                nc.scalar.activation(yst[:], ywin[:], ActF.Copy)
                nc.sync.dma_start(y_d[:, s * C:(s + 1) * C], yst[:])
                ysq = msb.tile([WIN, C], F32, tag="ysq")
                nc.vector.tensor_tensor(out=ysq[:], in0=yst[:], in1=yst[:],
                                        op=Alu.mult)
                if STAGE >= 4:
                    nc.tensor.matmul(out=stat_sum[:], lhsT=ones_col[:],
                                     rhs=yst[:], start=(s == 0), stop=(s == NW - 1))
                    nc.tensor.matmul(out=stat_sq[:], lhsT=ones_col[:],
                                     rhs=ysq[:], start=(s == 0), stop=(s == NW - 1))

            stat_sb = sb.tile([1, 2 * C], F32, tag="statsb")
            nc.vector.tensor_copy(stat_sb[:, 0:C], stat_sum[:])
            nc.vector.tensor_copy(stat_sb[:, C:2 * C], stat_sq[:])
            nc.sync.dma_start(stats_d[:], stat_sb[:])

            if with_norm_out:
                b_in = dramp.tile([1, 2 * C], F32)
                b_out = dramp.tile([1, 2 * C], F32)
                nc.sync.dma_start(b_in[:], stat_sb[:])
                nc.gpsimd.collective_compute(
                    "AllReduce", Alu.add,
                    replica_groups=[list(range(ncores))],
                    ins=[b_in[:]], outs=[b_out[:]],
                )
                sall = sb.tile([1, 2 * C], F32, tag="sall")
                nc.sync.dma_start(sall[:], b_out[:])
                gbt = sb.tile([1, 4 * C], F32, tag="gbt")
                nc.sync.dma_start(gbt[:], gb_d[:])
                invN = 1.0 / float(NTOT)
                mu = sb.tile([1, C], F32, tag="mu")
                nc.vector.tensor_scalar(out=mu[:], in0=sall[0:1, 0:C],
                                        scalar1=invN, scalar2=None, op0=Alu.mult)
                ex2 = sb.tile([1, C], F32, tag="ex2")
                nc.vector.tensor_scalar(out=ex2[:], in0=sall[0:1, C:2 * C],
                                        scalar1=invN, scalar2=None, op0=Alu.mult)
                musq = sb.tile([1, C], F32, tag="musq")
                nc.vector.tensor_tensor(out=musq[:], in0=mu[:], in1=mu[:],
                                        op=Alu.mult)
                var = sb.tile([1, C], F32, tag="var")
                nc.vector.tensor_tensor(out=var[:], in0=ex2[:], in1=musq[:],
                                        op=Alu.subtract)
                epst = sb.tile([1, 1], F32, tag="epst")
                nc.gpsimd.memset(epst[:], EPS)
                vare = sb.tile([1, C], F32, tag="vare")
                nc.vector.tensor_scalar(out=vare[:], in0=var[:],
                                        scalar1=epst[0:1, 0:1], scalar2=None,
                                        op0=Alu.add)
                sd = sb.tile([1, C], F32, tag="sd")
                nc.scalar.activation(sd[:], vare[:], ActF.Sqrt)
                rstd = sb.tile([1, C], F32, tag="rstd")
                nc.vector.reciprocal(rstd[:], sd[:])
                a_c = sb.tile([1, C], F32, tag="a_c")
                nc.vector.tensor_tensor(out=a_c[:], in0=rstd[:],
                                        in1=gbt[0:1, 0:C], op=Alu.mult)
                mua = sb.tile([1, C], F32, tag="mua")
                nc.vector.tensor_tensor(out=mua[:], in0=mu[:], in1=a_c[:],
                                        op=Alu.mult)
                b_c = sb.tile([1, C], F32, tag="b_c")
                nc.vector.tensor_tensor(out=b_c[:], in0=gbt[0:1, C:2 * C],
                                        in1=mua[:], op=Alu.subtract)
                ones_row = constp.tile([1, 128], F32)
                nc.gpsimd.memset(ones_row[:], 1.0)
                a_rep = constp.tile([128, C], F32)
                b_rep = constp.tile([128, C], F32)
                abp = ypp.tile([128, C], F32, tag="ywin")
                nc.tensor.matmul(out=abp[:], lhsT=ones_row[:], rhs=a_c[:],
                                 start=True, stop=True)
                nc.scalar.activation(a_rep[:], abp[:], ActF.Copy)
                abp2 = ypp.tile([128, C], F32, tag="ywin")
                nc.tensor.matmul(out=abp2[:], lhsT=ones_row[:], rhs=b_c[:],
                                 start=True, stop=True)
                nc.scalar.activation(b_rep[:], abp2[:], ActF.Copy)

                for s in range(NW):
                    yt = sb.tile([128, C], F32, tag="yt")
                    nc.sync.dma_start(yt[:], y_d[:, s * C:(s + 1) * C])
                    xrt = sb.tile([128, C], F32, tag="xrt")
                    nc.sync.dma_start(xrt[:], xr_d[:, s * C:(s + 1) * C])
                    t1 = sb.tile([128, C], F32, tag="t1")
                    nc.vector.tensor_tensor(out=t1[:], in0=yt[:], in1=a_rep[:],
                                            op=Alu.mult)
                    t2 = sb.tile([128, C], F32, tag="t2")
                    nc.vector.tensor_tensor(out=t2[:], in0=t1[:], in1=b_rep[:],
                                            op=Alu.add)
                    t3 = sb.tile([128, C], F32, tag="t3")
                    nc.vector.tensor_tensor(out=t3[:], in0=t2[:], in1=xrt[:],
                                            op=Alu.add)
                    t4 = sb.tile([128, C], F32, tag="t4")
                    nc.scalar.activation(t4[:], t3[:], ActF.Relu)
                    nc.sync.dma_start(out_d[:, s * C:(s + 1) * C], t4[:])

    nc.compile()
    return nc


def _lane_major_rows(arr_lm, nblk):
    """[128, nblk*64] lane-major -> [nblk*128, 64] rows."""
    return arr_lm.reshape(128, nblk, C).transpose(1, 0, 2).reshape(nblk * 128, C)


def _rows_to_lane_major(rows, nblk):
    return np.ascontiguousarray(
        rows.reshape(nblk, 128, C).transpose(1, 0, 2).reshape(128, nblk * C))


def kernel(x, in_idx, out_idx, W1, W2, gamma1, beta1, gamma2, beta2,
           profile=False):
    from concourse.bass_utils import run_bass_kernel_spmd

    x = np.asarray(x, np.float32)
    in_idx = np.asarray(in_idx)
    out_idx = np.asarray(out_idx)
    W1 = np.asarray(W1, np.float32)
    W2 = np.asarray(W2, np.float32)
    g1 = np.asarray(gamma1, np.float32)
    b1 = np.asarray(beta1, np.float32)
    g2 = np.asarray(gamma2, np.float32)
    b2 = np.asarray(beta2, np.float32)

    ii = in_idx.reshape(-1).astype(np.int64)
    oo = out_idx.reshape(-1).astype(np.int64)
    kf = np.repeat(np.arange(K, dtype=np.int64), in_idx.shape[1])
    core, win, loc = _route_conv(oo, kf)

    iota = np.broadcast_to(np.arange(128, dtype=np.float32),
                           (128, 128)).astype(BF16).copy()

    # ---- conv1 ----
    rows1 = x[ii]
    cores1, B1 = _build_stream(rows1, core, win, loc, kf, W1)
    wt1 = _w_table(W1)
    prog_key = ("A", B1, NW)
    if prog_key not in _cache:
        _cache[prog_key] = _build_program(B1, NW, N, with_norm_out=False)
    ncA = _cache[prog_key]
    in_maps = [{"stream": s, "oiT": o, "wt": wt1, "iota": iota}
               for (s, o) in cores1]
    import time as _t
    _t0 = _t.time()
    resA = run_bass_kernel_spmd(ncA, in_maps, core_ids=list(range(NCORES)),
                                trace=profile)
    kernel._runA_s = _t.time() - _t0
    y1 = np.zeros((NCORES * PADROWS, C), np.float32)
    stats1 = np.zeros((2, C), np.float64)
    for c in range(NCORES):
        y1[c * PADROWS:(c + 1) * PADROWS] = _lane_major_rows(
            resA.results[c]["y"], NW)
        stats1 += resA.results[c]["stats"].reshape(2, C).astype(np.float64)

    # norm1 on host (builds conv2 stream input h)
    mu1 = (stats1[0] / N).astype(np.float32)
    var1 = (stats1[1] / N).astype(np.float32) - mu1 * mu1
    a1 = (g1 / np.sqrt(var1 + EPS)).astype(np.float32)
    bb1 = (b1 - mu1 * a1).astype(np.float32)
    # gather h rows for conv2: h[i] = relu(a1*y1[i] + bb1)
    ii_pad = (ii // SHARD) * PADROWS + (ii % SHARD)
    rows2 = np.maximum(y1[ii_pad] * a1 + bb1, 0.0)

    # ---- conv2 ----
    cores2, B2 = _build_stream(rows2, core, win, loc, kf, W2)
    wt2 = _w_table(W2)
    prog_key2 = ("B", B2, NW)
    if prog_key2 not in _cache:
        _cache[prog_key2] = _build_program(B2, NW, N, with_norm_out=True)
    ncB = _cache[prog_key2]
    gb = np.concatenate([g2, b2, np.zeros_like(g2), np.zeros_like(b2)])[None, :]
    in_maps2 = []
    for c in range(NCORES):
        xr = np.zeros((PADROWS, C), np.float32)
        xr[:SHARD] = x[c * SHARD:(c + 1) * SHARD]
        in_maps2.append({
            "stream": cores2[c][0], "oiT": cores2[c][1], "wt": wt2,
            "iota": iota, "xr": _rows_to_lane_major(xr, NW), "gb": gb,
        })
    _t0 = _t.time()
    resB = run_bass_kernel_spmd(ncB, in_maps2, core_ids=list(range(NCORES)),
                                trace=profile)
    kernel._runB_s = _t.time() - _t0
    out = np.zeros((N, C), np.float32)
    for c in range(NCORES):
        rows = _lane_major_rows(resB.results[c]["out"], NW)
        out[c * SHARD:(c + 1) * SHARD] = rows[:SHARD]
    kernel._last = (resA, resB)
    return out

